# revision 6
# baseline (speedup 1.0000x reference)
"""Causal single-head attention (B=4, T=4096, E=1024, D=64) on 8 trn2 NeuronCores.

Strategy:
  - 2 cores per batch.  Per batch, query rows are split causally-balanced:
      "outer"  core: rows [0:1024) u [3072:4096)   (512-chunks 0,1,6,7)
      "middle" core: rows [1024:3072)              (512-chunks 2,3,4,5)
    Both halves do an identical amount of score/AV work (72 k-blocks of 128).
  - No collectives: each core projects k/v for the full causal range it needs
    (outer: 4096 rows, middle: 3072 rows) from a host-pre-transposed x.
  - Host pre-transposes x (xT = x[b].T), so all projections read xT directly;
    no on-chip transposes of x are needed.
  - Flash-style: scores are computed transposed (S^T[tk, tq]) so the softmax
    denominator comes for free as a 65th "ones" column of v in the AV matmul,
    and no row-max pass is needed (scores are ~N(0, 0.083^2), exp is safe).
  - kT is stored partition-folded (even 512-chunks on partitions 0:64, odd on
    64:128) and qT is duplicated on both halves, so score matmuls (K=64) run
    pairwise-concurrent on the PE via row tiling.
  - Matmuls run as float32r (full PE rate at N=512; data stays fp32 in SBUF).

Two programs (outer/middle), each run 4-core SPMD on a disjoint device set,
dispatched concurrently.
"""

import functools

import numpy as np

import concourse.bass as bass
import concourse.mybir as mybir
import concourse.tile as tile
from concourse import bacc
from concourse.masks import make_identity

E = 1024
D = 64
T = 4096
B = 4
CH = 512  # column chunk for matmul streaming (fp32 moving-operand max)
NB = 128  # tk block size (PE partition dim)
SCALE = 1.0 / 32.0  # E ** -0.5

OUTER_GIDS = (0, 1, 6, 7)  # global 512-row q-chunks handled by the outer core
MIDDLE_GIDS = (2, 3, 4, 5)
OUTER_NKV = 8  # kv range in 512-chunks (outer needs all 4096 rows)
MIDDLE_NKV = 6  # middle needs rows [0:3072)

FP32 = mybir.dt.float32
FP32R = mybir.dt.float32r
MM_DT = FP32R  # matmul compute dtype (flip to FP32 if fp32r numerics fail)


def _build_body(ctx, tc, xT, wk, wv, wq, out, n_kv, q_gids, mm_dt):
    nc = tc.nc
    L = n_kv * CH
    n_blocks = L // NB
    nq_chunks = len(q_gids)

    pers = ctx.enter_context(tc.tile_pool(name="pers", bufs=1))
    xc_pool = ctx.enter_context(tc.tile_pool(name="xc", bufs=16))
    stage = ctx.enter_context(tc.tile_pool(name="stage", bufs=2))
    exp_pool = ctx.enter_context(tc.tile_pool(name="expp", bufs=4))
    sm_pool = ctx.enter_context(tc.tile_pool(name="sm", bufs=2))
    ps_big = ctx.enter_context(tc.tile_pool(name="ps_big", bufs=4, space="PSUM"))
    ps_acc = ctx.enter_context(tc.tile_pool(name="ps_acc", bufs=2, space="PSUM"))
    ps_sm = ctx.enter_context(tc.tile_pool(name="ps_sm", bufs=2, space="PSUM"))

    # ---- persistent SBUF tensors ----
    wa_sb = pers.tile([128, E], mm_dt, tag="wa")  # [Wk|Wv] per e-block
    wb_sb = pers.tile([128, E], mm_dt, tag="wb")  # [Wv|Wk] per e-block
    wq_sb = pers.tile([128, E], mm_dt, tag="wq2")  # [Wq|Wq] per e-block
    kt2 = pers.tile([128, L // 2], mm_dt, tag="kt2")  # folded kT
    qtd = pers.tile([128, nq_chunks * CH], mm_dt, tag="qtd")  # duplicated qT
    v_sb = pers.tile([128, n_blocks * (D + 1)], mm_dt, tag="vsb")  # [v | 1] blocks
    masks = pers.tile([128, 4 * CH], FP32, tag="masks")
    ident = pers.tile([128, 128], FP32, tag="ident")

    # identity (for PE transposes)
    make_identity(nc, ident[:])

    # staircase causal masks M_j[r, c] = 1 iff c - r - 128*j >= 0
    nc.gpsimd.memset(masks[:], 1.0)
    for j in range(4):
        nc.gpsimd.affine_select(
            out=masks[:, CH * j : CH * (j + 1)],
            in_=masks[:, CH * j : CH * (j + 1)],
            compare_op=mybir.AluOpType.is_ge,
            fill=0.0,
            base=-NB * j,
            channel_multiplier=-1,
            pattern=[[1, CH]],
        )

    # v ones-columns via one strided ACT copy (memset can't write fp32r)
    ones_view = v_sb[:].rearrange("p (b c) -> p b c", c=D + 1)[:, :, D : D + 1]
    const1 = nc.const_aps.tensor(1.0, (128, n_blocks, 1), FP32)
    nc.scalar.activation(
        ones_view, const1, mybir.ActivationFunctionType.Copy, bias=0.0, scale=1.0
    )

    # ---- load weights into stacked SBUF layout ----
    # DRAM w [E, D] -> per e-block eb, sbuf col range [128*eb + off, +64)
    def load_w(dst, w_dram, off):
        src = w_dram.rearrange("(e p) m -> p e m", p=128)
        dst3 = dst[:].rearrange("p (e m) -> p e m", e=8)
        nc.sync.dma_start(dst3[:, :, off : off + D], src)

    load_w(wa_sb, wk, 0)
    load_w(wa_sb, wv, D)
    load_w(wb_sb, wv, 0)
    load_w(wb_sb, wk, D)
    load_w(wq_sb, wq, 0)
    load_w(wq_sb, wq, D)

    # ---- projection over kv chunks ----
    q_local = {g: i for i, g in enumerate(q_gids)}
    for c in range(n_kv):
        xc = []
        for eb in range(8):
            t = xc_pool.tile([128, CH], mm_dt, tag="xc")
            nc.sync.dma_start(t[:], xT[128 * eb : 128 * (eb + 1), CH * c : CH * (c + 1)])
            xc.append(t)

        wstack = wa_sb if c % 2 == 0 else wb_sb
        khalf = 0 if c % 2 == 0 else 64  # partition base of kT in psum
        vhalf = 64 - khalf

        kv_ps = ps_big.tile([128, CH], FP32, tag="psbig")
        for eb in range(8):
            nc.tensor.matmul(
                kv_ps[:],
                wstack[:, 128 * eb : 128 * (eb + 1)],
                xc[eb][:],
                start=(eb == 0),
                stop=(eb == 7),
            )

        # kT chunk -> folded storage (lane-aligned copy)
        kcols = slice(CH * (c // 2), CH * (c // 2) + CH)
        nc.scalar.copy(kt2[khalf : khalf + 64, kcols], kv_ps[khalf : khalf + 64, :])

        # vT chunk -> stage -> PE transpose -> v_sb (natural layout)
        vst = stage.tile([128, CH], FP32, tag="vst")
        nc.vector.tensor_copy(vst[vhalf : vhalf + 64, :], kv_ps[vhalf : vhalf + 64, :])
        for j in range(4):
            blk = 4 * c + j
            vt_ps = ps_sm.tile([128, D + 1], FP32, tag="pssm")
            nc.tensor.transpose(
                vt_ps[:, 0:D],
                vst[vhalf : vhalf + 64, NB * j : NB * (j + 1)],
                ident[vhalf : vhalf + 64, vhalf : vhalf + 64],
            )
            nc.vector.tensor_copy(
                v_sb[:, (D + 1) * blk : (D + 1) * blk + D], vt_ps[:, 0:D]
            )

        if c in q_local:
            qi = q_local[c]
            q_ps = ps_big.tile([128, CH], FP32, tag="psbig")
            for eb in range(8):
                nc.tensor.matmul(
                    q_ps[:],
                    wq_sb[:, 128 * eb : 128 * (eb + 1)],
                    xc[eb][:],
                    start=(eb == 0),
                    stop=(eb == 7),
                )
            nc.scalar.copy(qtd[:, CH * qi : CH * (qi + 1)], q_ps[:])

    # ---- attention per q-chunk ----
    for qi, g_chunk in enumerate(q_gids):
        ntk = 4 * (g_chunk + 1)  # tk blocks 0..ntk-1
        qcols = slice(CH * qi, CH * (qi + 1))

        ev = [g for g in range(ntk) if (g // 4) % 2 == 0]
        od = [g for g in range(ntk) if (g // 4) % 2 == 1]
        pairs = []
        for i in range(max(len(ev), len(od))):
            p = []
            if i < len(ev):
                p.append(ev[i])
            if i < len(od):
                p.append(od[i])
            pairs.append(p)

        acc = ps_acc.tile([D + 1, CH], FP32, tag="psacc")
        n_mm = ntk
        mm_i = 0

        def emit_av(items):
            nonlocal mm_i
            for g, e_t in items:
                nc.tensor.matmul(
                    acc[:],
                    v_sb[:, (D + 1) * g : (D + 1) * (g + 1)],
                    e_t[:],
                    start=(mm_i == 0),
                    stop=(mm_i == n_mm - 1),
                )
                mm_i += 1

        prev = None
        for pair in pairs:
            cur = []
            for g in pair:
                half = 64 * ((g // 4) % 2)
                kc = g // 4  # kv chunk of this block
                kcol0 = CH * (kc // 2) + NB * (g % 4)
                s_ps = ps_big.tile([128, CH], FP32, tag="psbig")
                nc.tensor.matmul(
                    s_ps[:],
                    kt2[half : half + 64, kcol0 : kcol0 + NB],
                    qtd[half : half + 64, qcols],
                    start=True,
                    stop=True,
                )
                e_t = exp_pool.tile([128, CH], mm_dt, tag="expt")
                nc.scalar.activation(
                    e_t[:], s_ps[:], mybir.ActivationFunctionType.Exp,
                    bias=0.0, scale=SCALE,
                )
                j = g - 4 * g_chunk
                if j >= 0:
                    nc.vector.tensor_mul(
                        e_t[:], e_t[:], masks[:, CH * j : CH * (j + 1)]
                    )
                cur.append((g, e_t))
            if prev is not None:
                emit_av(prev)
            prev = cur
        emit_av(prev)

        # epilogue: transpose outT [65, 512] -> [512, 65], normalize, store
        ot = sm_pool.tile([D + 1, CH], FP32, tag="ot")
        nc.scalar.copy(ot[:], acc[:])
        osb = sm_pool.tile([128, 4 * D], FP32, tag="osb")
        for j in range(4):
            o_ps = ps_sm.tile([128, D + 1], FP32, tag="pssm")
            nc.tensor.transpose(
                o_ps[:], ot[:, NB * j : NB * (j + 1)], ident[0 : D + 1, 0 : D + 1]
            )
            r = sm_pool.tile([128, 1], FP32, tag="recip")
            nc.vector.reciprocal(r[:], o_ps[:, D : D + 1])
            nc.vector.tensor_scalar_mul(osb[:, D * j : D * (j + 1)], o_ps[:, 0:D], r[:])
        dst = out[CH * qi : CH * (qi + 1), :].rearrange("(j p) d -> p j d", p=128)
        nc.sync.dma_start(dst, osb[:].rearrange("p (j d) -> p j d", j=4))


def build_program(n_kv, q_gids, mm_dt=MM_DT, num_devices=4):
    import contextlib

    nc = bacc.Bacc(
        "TRN2", target_bir_lowering=False, debug=False, num_devices=num_devices
    )
    L = n_kv * CH
    nq = len(q_gids) * CH
    xT = nc.dram_tensor("xT", [E, L], mm_dt, kind="ExternalInput").ap()
    wk = nc.dram_tensor("wk", [E, D], mm_dt, kind="ExternalInput").ap()
    wv = nc.dram_tensor("wv", [E, D], mm_dt, kind="ExternalInput").ap()
    wq = nc.dram_tensor("wq", [E, D], mm_dt, kind="ExternalInput").ap()
    out = nc.dram_tensor("out", [nq, D], FP32, kind="ExternalOutput").ap()
    with tile.TileContext(nc) as tc:
        with contextlib.ExitStack() as ctx:
            _build_body(ctx, tc, xT, wk, wv, wq, out, n_kv, q_gids, mm_dt)
    nc.compile()
    return nc


# ---------------- host-side runner ----------------


def _make_runner(nc, devices):
    import jax
    from jax.experimental.shard_map import shard_map
    from jax.sharding import Mesh, PartitionSpec

    from concourse import bass2jax

    bass2jax.install_neuronx_cc_hook()

    fn0 = nc.m.functions[0]
    partition_name = nc.partition_id_tensor.name if nc.partition_id_tensor else None
    in_names, out_names, out_avals = [], [], []
    for alloc in fn0.allocations:
        if not isinstance(alloc, mybir.MemoryLocationSet):
            continue
        if alloc.kind not in ("ExternalInput", "ExternalOutput"):
            continue
        name = alloc.memorylocations[0].name
        if alloc.kind == "ExternalInput":
            if name != partition_name:
                in_names.append(name)
        else:
            out_names.append(name)
            out_avals.append(
                jax.core.ShapedArray(
                    tuple(alloc.tensor_shape), mybir.dt.np(alloc.dtype)
                )
            )
    n_params = len(in_names)
    n_outs = len(out_names)
    all_names = list(in_names) + list(out_names)
    if partition_name is not None:
        all_names.append(partition_name)
    all_names = tuple(all_names)

    def _body(*args):
        operands = list(args)
        if partition_name is not None:
            operands.append(bass2jax.partition_id_tensor())
        outs = bass2jax._bass_exec_p.bind(
            *operands,
            out_avals=tuple(out_avals),
            in_names=all_names,
            out_names=tuple(out_names),
            lowering_input_output_aliases=(),
            sim_require_finite=True,
            sim_require_nnan=True,
            nc=nc,
        )
        return tuple(outs)

    n_cores = len(devices)
    mesh = Mesh(np.asarray(devices), ("core",))
    in_specs = (PartitionSpec("core"),) * (n_params + n_outs)
    out_specs = (PartitionSpec("core"),) * n_outs
    donate = tuple(range(n_params, n_params + n_outs))
    sharded = jax.jit(
        shard_map(
            _body, mesh=mesh, in_specs=in_specs, out_specs=out_specs, check_rep=False
        ),
        donate_argnums=donate,
        keep_unused=True,
    )
    return {
        "fn": sharded,
        "in_names": in_names,
        "out_names": out_names,
        "out_avals": out_avals,
        "n_cores": n_cores,
    }


@functools.lru_cache(maxsize=1)
def _get_programs():
    import jax

    devs = jax.devices()
    assert len(devs) >= 8, f"need 8 neuron cores, have {devs}"
    nc_outer = build_program(OUTER_NKV, OUTER_GIDS)
    nc_middle = build_program(MIDDLE_NKV, MIDDLE_GIDS)
    run_outer = _make_runner(nc_outer, devs[0:4])
    run_middle = _make_runner(nc_middle, devs[4:8])
    return run_outer, run_middle


def _concat_inputs(runner, per_core_maps):
    arrs = []
    for name in runner["in_names"]:
        arrs.append(np.concatenate([m[name] for m in per_core_maps], axis=0))
    for av in runner["out_avals"]:
        arrs.append(np.zeros((runner["n_cores"] * av.shape[0], *av.shape[1:]), av.dtype))
    return arrs


def _split_outputs(runner, out_arrs):
    res = []
    for c in range(runner["n_cores"]):
        m = {}
        for i, name in enumerate(runner["out_names"]):
            shp = runner["out_avals"][i].shape
            m[name] = np.asarray(out_arrs[i]).reshape(
                runner["n_cores"], *shp
            )[c]
        res.append(m)
    return res


def make_core_inputs(x, Wq, Wk, Wv):
    """Build per-core input maps for the outer (4) and middle (4) programs."""
    x = np.asarray(x, dtype=np.float32)
    Wq = np.asarray(Wq, dtype=np.float32)
    Wk = np.asarray(Wk, dtype=np.float32)
    Wv = np.asarray(Wv, dtype=np.float32)
    outer_maps, middle_maps = [], []
    for b in range(B):
        xTb = np.ascontiguousarray(x[b].T)  # [E, T]
        outer_maps.append({"xT": xTb, "wk": Wk, "wv": Wv, "wq": Wq})
        middle_maps.append(
            {"xT": np.ascontiguousarray(xTb[:, : MIDDLE_NKV * CH]),
             "wk": Wk, "wv": Wv, "wq": Wq}
        )
    return outer_maps, middle_maps


def assemble_output(outer_res, middle_res):
    out = np.empty((B, T, D), dtype=np.float32)
    for b in range(B):
        oc = outer_res[b]["out"]
        mc = middle_res[b]["out"]
        for qi, g in enumerate(OUTER_GIDS):
            out[b, CH * g : CH * (g + 1)] = oc[CH * qi : CH * (qi + 1)]
        for qi, g in enumerate(MIDDLE_GIDS):
            out[b, CH * g : CH * (g + 1)] = mc[CH * qi : CH * (qi + 1)]
    return out


def kernel(x, Wq, Wk, Wv):
    run_outer, run_middle = _get_programs()
    outer_maps, middle_maps = make_core_inputs(x, Wq, Wk, Wv)
    a_in = _concat_inputs(run_outer, outer_maps)
    b_in = _concat_inputs(run_middle, middle_maps)
    a_out = run_outer["fn"](*a_in)  # async dispatch
    b_out = run_middle["fn"](*b_in)
    outer_res = _split_outputs(run_outer, a_out)
    middle_res = _split_outputs(run_middle, b_out)
    return assemble_output(outer_res, middle_res)


if __name__ == "__main__":
    rng = np.random.default_rng(0)
    x = rng.standard_normal((B, T, E), dtype=np.float32)
    s = 1.0 / np.sqrt(E)
    Wq = rng.uniform(-s, s, (E, D)).astype(np.float32)
    Wk = rng.uniform(-s, s, (E, D)).astype(np.float32)
    Wv = rng.uniform(-s, s, (E, D)).astype(np.float32)
    out = kernel(x, Wq, Wk, Wv)
    print("out", out.shape, out.dtype, np.abs(out).mean())
